# revision 9
# baseline (speedup 1.0000x reference)
"""MemNN (end-to-end memory network) Trainium2 kernel.

All the heavy FLOPs of this network are six (B*L, V) @ (V, D) embedding
matmuls that share `facts` as LHS (A_h = facts @ Wa[h], C_h = facts @ Wc[h],
h = 0..2), plus one question embedding.  The six fuse into a single
(3200, 10000) @ (10000, 1536) matmul that does NOT depend on the hop
recurrence, so the whole 98.3 GFLOP is one bulk matmul.

Sharding: vocab (contraction) dim split 8 ways -> each core reads only its
1/8 slice of facts/Wa/Wc/Wq, computes a partial product, and writes it to
DRAM.  The host unshards by summing the 8 partials and runs the tiny
sequential hop recurrence (~0.03% of total FLOPs) in fp32.

Numerics: inputs are cast to fp16 on the host (matmul inputs; PSUM
accumulation is fp32; partial outputs are written as fp16 and summed in
fp32 on the host).  Measured end-to-end rel err ~9e-4 vs the fp32
reference, comfortably under the 2e-2 gate.  fp16 (vs fp32r) enables
fast-weight-load, halves DMA traffic, and removes the <256-moving-dim
penalty.

Schedule: facts for the core stay fully SBUF-resident (64 KB/partition),
DMAed k-slice-major so compute can start after the first slice.  The main
loop is n (12 output tiles) -> k (10 contraction tiles) -> mi (8 moving
chunks of 400): one stationary tile serves 8 consecutive matmuls, and the 8
moving chunks accumulate into all 8 PSUM banks.  The last k iteration
interleaves each chunk's PSUM drain right after its closing matmul,
alternating DVE/Activation engines, so bank reuse in the next n-group never
stalls the PE.
"""

import os

os.environ.setdefault("MYCRO_LOCAL_CACHE", "1")

import numpy as np

import concourse.bass as bass
import concourse.mybir as mybir
import concourse.tile as tile
from concourse.bass_utils import run_bass_kernel_spmd

HOPS, B, L, V, D = 3, 64, 50, 10000, 256
NCORES = 8
BL = B * L                # 3200 moving rows
NF = 2 * HOPS * D         # 1536 fused output cols: [Wa0|Wa1|Wa2|Wc0|Wc1|Wc2]
VSH = V // NCORES         # 1250 vocab rows per core
KT = 10                   # contraction tiles of 128 per core
VPAD = KT * 128           # 1280 (zero-padded)
MCH = 400                 # moving-col chunk: (128, 400) fp32 PSUM = 1 bank
NM = BL // MCH            # 8 chunks -> exactly the 8 PSUM banks
NN = NF // 128            # 12 stationary W tiles
F16 = mybir.dt.float16
F8 = mybir.dt.float8e3   # e3m4: 4 mantissa bits
U8 = mybir.dt.uint8      # fp8 bytes travel as uint8 (axon PJRT rejects f8 buffers)
F32 = mybir.dt.float32

_nc_cache = None
_last_result = None       # BassKernelResults of the most recent run (for profiling)


def _legalize_sync(nc):
    """Split multi-wait sync_info into standalone single-wait EventSemaphores.

    The walrus build in this environment enforces the raw-bass contract of at
    most ONE SyncWait per instruction ("Too many sync wait commands" in
    setupSyncWait otherwise), while Tile attaches every needed wait to the
    consuming instruction.  Hoisting all-but-one wait onto preceding
    InstEventSemaphore instructions on the same engine queue is semantically
    identical: engine queues are in-order, so a preceding wait blocks the
    queue exactly like an attached wait.  Updates are left untouched (they
    fire at completion and cannot be hoisted).
    """
    for func in nc.m.functions:
        for block in func.blocks:
            insts = list(block.instructions)
            out = []
            n = 0
            for inst in insts:
                si = inst.sync_info
                if si is not None and len(si.on_wait) > 1:
                    waits = list(si.on_wait)
                    for w in waits[:-1]:
                        ev = mybir.InstEventSemaphore(
                            name=f"{inst.name}-hoistw{n}", ins=[], outs=[]
                        )
                        n += 1
                        ev.engine = inst.engine
                        ev.sync_info = mybir.SyncInfo(on_wait=[w], on_update=[])
                        nc.register_instruction(ev)
                        out.append(ev)
                    inst.sync_info = mybir.SyncInfo(
                        on_wait=[waits[-1]], on_update=list(si.on_update)
                    )
                out.append(inst)
            if len(out) != len(insts):
                block.instructions = out
    return nc


def _dedup_ldweights(nc):
    """Drop InstLdweights that reload the exact weights already in the PE.

    The Tile pipeline splits every matmul into (InstLdweights, InstMatmult
    [non-self-loading]) and emits one Ldweights per matmul even when
    consecutive matmuls share the same stationary tile.  Weights persist in
    the array across matmuls, so a Ldweights whose access pattern equals the
    previous one on the same queue is a pure ~128-cycle waste on the PE's
    critical path (measured ~336 vs ~200 cyc/matmul at 400 moving cols).
    Any sync waits/updates on a dropped Ldweights are merged into the next
    instruction so the Tile dependency tracking stays intact.
    """

    def key(inst):
        ap = inst.ins[0]
        return (
            ap.memref,
            ap.offset,
            str(ap.ap),
            str(ap.dtype),
            getattr(inst, "is_transpose", None),
            getattr(inst, "perf_mode", None),
        )

    dropped = 0
    for func in nc.m.functions:
        for block in func.blocks:
            insts = list(block.instructions)
            out = []
            last_key = None
            pending_sync = []  # sync_infos of dropped ldweights
            for inst in insts:
                if isinstance(inst, mybir.InstLdweights):
                    k = key(inst)
                    if k == last_key:
                        if inst.sync_info is not None:
                            pending_sync.append(inst.sync_info)
                        dropped += 1
                        continue
                    last_key = k
                elif isinstance(inst, mybir.InstMatmult):
                    if getattr(inst, "ldweights", False):
                        last_key = None  # self-loading matmul clobbers array
                if pending_sync:
                    waits = list(inst.sync_info.on_wait) if inst.sync_info else []
                    updates = list(inst.sync_info.on_update) if inst.sync_info else []
                    for si in pending_sync:
                        waits.extend(si.on_wait)
                        updates.extend(si.on_update)
                    inst.sync_info = mybir.SyncInfo(on_wait=waits, on_update=updates)
                    pending_sync = []
                out.append(inst)
            assert not pending_sync, "dropped ldweights sync with no successor"
            if len(out) != len(insts):
                block.instructions = out
    return dropped


def _build(reps=1):
    """Build the SPMD device program.

    reps>1 repeats the main loop body (same data, same output addresses) —
    used only by the benchmark harness to measure device time differentially
    (per-call dispatch noise over the axon tunnel is ~ms, device time is
    ~200 us, so wall-clocking one launch cannot resolve it).
    """
    nc = bass.Bass(trn_type="TRN2")
    facts_t = nc.dram_tensor("facts_t", [VPAD, BL], U8, kind="ExternalInput")
    wac = nc.dram_tensor("wac", [VPAD, NF], F16, kind="ExternalInput")
    q_t = nc.dram_tensor("q_t", [VPAD, B], F16, kind="ExternalInput")
    wq = nc.dram_tensor("wq", [VPAD, D], F16, kind="ExternalInput")
    pac_t = nc.dram_tensor("pac_t", [NF, BL], F16, kind="ExternalOutput")
    pu = nc.dram_tensor("pu", [B, D], F32, kind="ExternalOutput")

    fr = facts_t.rearrange("(k p) n -> p k n", p=128)
    wr = wac.rearrange("(k p) n -> p k n", p=128)
    qr = q_t.rearrange("(k p) n -> p k n", p=128)
    wqr = wq.rearrange("(k p) n -> p k n", p=128)

    with (
        tile.TileContext(nc) as tc,
        tc.tile_pool(name="wpool", bufs=1) as wpool,
        tc.tile_pool(name="opool", bufs=8) as opool,
        tc.tile_pool(name="pspool", bufs=8, space="PSUM") as pspool,
    ):
        # Prologue DMA order: the n=0 wac slice (all k), then facts k-slice
        # major (k=0 unblocks the first 8 matmuls), then the rest of wac,
        # then the small question tensors.
        wt = wpool.tile([128, KT, NF], F16)
        nc.sync.dma_start(wt[:, :, 0:128], wr[:, :, 0:128])
        xt = wpool.tile([128, KT, BL], U8)
        for k in range(KT):
            nc.sync.dma_start(xt[:, k, :], fr[:, k, :])
        for off in range(128, NF, 512):
            end = min(off + 512, NF)
            nc.sync.dma_start(wt[:, :, off:end], wr[:, :, off:end])
        qtile = wpool.tile([128, KT, B], F16)
        nc.sync.dma_start(qtile[:], qr)
        wqt = wpool.tile([128, KT, D], F16)
        nc.sync.dma_start(wqt[:], wqr)

        # Main fused matmul: out(n, m) += sum_k wac[k, n].T @ facts_t[k, m].
        # One stationary tile (k, n) feeds all 8 moving chunks; the 8 chunks
        # of one n-group occupy all 8 PSUM banks; drains interleave with the
        # closing k=KT-1 matmuls, alternating DVE/ACT.
        for _ in range(reps):
            for n in range(NN):
                pss = [
                    pspool.tile([128, MCH], F32, tag="ps", name="ps")
                    for _ in range(NM)
                ]
                for k in range(KT):
                    last = k == KT - 1
                    for mi in range(NM):
                        nc.tensor.matmul(
                            pss[mi][:],
                            wt[:, k, n * 128 : (n + 1) * 128],
                            xt[:, k, mi * MCH : (mi + 1) * MCH].bitcast(F8),
                            start=(k == 0),
                            stop=last,
                        )
                        if last:
                            ot = opool.tile([128, MCH], F16, tag="ot", name="ot")
                            if mi % 2 == 0:
                                nc.vector.tensor_copy(ot[:], pss[mi][:])
                            else:
                                nc.scalar.copy(out=ot[:], in_=pss[mi][:])
                            nc.sync.dma_start(
                                pac_t[
                                    n * 128 : (n + 1) * 128,
                                    mi * MCH : (mi + 1) * MCH,
                                ],
                                ot[:],
                            )

        # Question embedding at the tail: its PE work (10 small matmuls)
        # overlaps the main loop's epilogue.
        psq = pspool.tile([B, D], F32, tag="ps", name="ps", padded_shape=[128, MCH])
        for k in range(KT):
            nc.tensor.matmul(
                psq[:], qtile[:, k, :], wqt[:, k, :], start=(k == 0), stop=(k == KT - 1)
            )
        uo = opool.tile([B, D], F32, tag="uo")
        nc.any.tensor_copy(out=uo[:], in_=psq[:])
        nc.sync.dma_start(pu[:, :], uo[:])
    _dedup_ldweights(nc)
    return _legalize_sync(nc)


def _shard_inputs(facts, question, Wq, Wa, Wc):
    fx = np.ascontiguousarray(facts, dtype=np.float32).reshape(BL, V)
    qx = np.asarray(question, dtype=np.float32).sum(axis=1)  # (B, V) bag-of-words
    Wq = np.asarray(Wq, dtype=np.float32)
    Wa = np.asarray(Wa, dtype=np.float32)
    Wc = np.asarray(Wc, dtype=np.float32)
    wac_full = np.concatenate([Wa[0], Wa[1], Wa[2], Wc[0], Wc[1], Wc[2]], axis=1)

    in_maps = []
    for c in range(NCORES):
        sl = slice(c * VSH, (c + 1) * VSH)
        import ml_dtypes
        ft = np.zeros((VPAD, BL), ml_dtypes.float8_e3m4)
        ft[:VSH] = (4.0 * fx[:, sl].T).astype(ml_dtypes.float8_e3m4)
        ft = ft.view(np.uint8)
        qt = np.zeros((VPAD, B), np.float16)
        qt[:VSH] = qx[:, sl].T
        ws = np.zeros((VPAD, NF), np.float16)
        ws[:VSH] = wac_full[sl]
        wqs = np.zeros((VPAD, D), np.float16)
        wqs[:VSH] = Wq[sl]
        in_maps.append({"facts_t": ft, "q_t": qt, "wac": ws, "wq": wqs})
    return in_maps


def _wait_for_devices(min_wait_attempts=10):
    """The axon terminal occasionally reports a transient bad topology
    ("terminal has 1 core"); poll until all 8 NeuronCores are visible."""
    import time as _time

    import jax

    for attempt in range(min_wait_attempts):
        try:
            if len(jax.devices()) >= NCORES:
                return
        except Exception:  # noqa: BLE001 - backend init failure is retryable
            try:
                jax.clear_backends()
            except Exception:  # noqa: BLE001
                pass
        _time.sleep(15.0)
    # fall through: let the run itself raise a descriptive error


def _run_with_retries(nc, in_maps, attempts=4):
    """run_bass_kernel_spmd with retries: the axon terminal occasionally
    reports transient failures (device wedged / NRT_EXEC_UNIT_UNRECOVERABLE /
    temporary topology glitches) that succeed on re-dispatch."""
    import time as _time

    last_exc = None
    for attempt in range(attempts):
        try:
            return run_bass_kernel_spmd(nc, in_maps, list(range(NCORES)))
        except Exception as e:  # noqa: BLE001 - retry any runtime failure
            last_exc = e
            if attempt < attempts - 1:
                _time.sleep(10.0 * (attempt + 1))
                _wait_for_devices(min_wait_attempts=4)
    raise last_exc


def kernel(facts, question, Wq, Wa, Wc, Ww, bw):
    global _nc_cache, _last_result
    _wait_for_devices(min_wait_attempts=8)
    in_maps = _shard_inputs(facts, question, Wq, Wa, Wc)
    if _nc_cache is None:
        _nc_cache = _build()
    _last_result = _run_with_retries(_nc_cache, in_maps)
    res = _last_result.results

    # Unshard: sum the 8 partial products of the vocab-sharded matmul.
    ac_t = res[0]["pac_t"].astype(np.float32)  # partials carry the 4x facts scale
    u = res[0]["pu"].copy()
    for r in res[1:]:
        ac_t += r["pac_t"].astype(np.float32)
        u += r["pu"]
    ac_t *= 0.25  # undo the 4x facts prescale

    # Sequential hop recurrence (tiny: ~30 MFLOP vs 98.3 GFLOP on device).
    Ww = np.asarray(Ww, dtype=np.float32)
    bw = np.asarray(bw, dtype=np.float32)
    for h in range(HOPS):
        A = ac_t[h * D : (h + 1) * D].reshape(D, B, L)
        C = ac_t[(HOPS + h) * D : (HOPS + h + 1) * D].reshape(D, B, L)
        match = np.einsum("dbl,bd->bl", A, u)
        mm = match - match.max(axis=-1, keepdims=True)
        e = np.exp(mm)
        p = e / e.sum(axis=-1, keepdims=True)
        att = np.einsum("bl,dbl->bd", p, C)
        z = (u + att) @ Ww[h] + bw[h]
        if h == HOPS - 1:
            zz = z - z.max(axis=-1, keepdims=True)
            ez = np.exp(zz)
            u = ez / ez.sum(axis=-1, keepdims=True)
        else:
            u = np.maximum(z, 0.0)
    return np.ascontiguousarray(u, dtype=np.float32)


# revision 10
# speedup vs baseline: 1.6178x; 1.6178x over previous
"""MemNN (end-to-end memory network) Trainium2 kernel.

All the heavy FLOPs of this network are six (B*L, V) @ (V, D) embedding
matmuls that share `facts` as LHS (A_h = facts @ Wa[h], C_h = facts @ Wc[h],
h = 0..2), plus one question embedding.  The six fuse into a single
(3200, 10000) @ (10000, 1536) matmul that does NOT depend on the hop
recurrence, so the whole 98.3 GFLOP is one bulk matmul.

Sharding: vocab (contraction) dim split 8 ways -> each core reads only its
1/8 slice of facts/Wa/Wc/Wq, computes a partial product, and writes it to
DRAM.  The host unshards by summing the 8 partials and runs the tiny
sequential hop recurrence (~0.03% of total FLOPs) in fp32.

Numerics: inputs are cast to fp16 on the host (matmul inputs; PSUM
accumulation is fp32; partial outputs are written as fp16 and summed in
fp32 on the host).  Measured end-to-end rel err ~9e-4 vs the fp32
reference, comfortably under the 2e-2 gate.  fp16 (vs fp32r) enables
fast-weight-load, halves DMA traffic, and removes the <256-moving-dim
penalty.

Schedule: facts for the core stay fully SBUF-resident (64 KB/partition),
DMAed k-slice-major so compute can start after the first slice.  The main
loop is n (12 output tiles) -> k (10 contraction tiles) -> mi (8 moving
chunks of 400): one stationary tile serves 8 consecutive matmuls, and the 8
moving chunks accumulate into all 8 PSUM banks.  The last k iteration
interleaves each chunk's PSUM drain right after its closing matmul,
alternating DVE/Activation engines, so bank reuse in the next n-group never
stalls the PE.
"""

import os

os.environ.setdefault("MYCRO_LOCAL_CACHE", "1")

import numpy as np

import concourse.bass as bass
import concourse.mybir as mybir
import concourse.tile as tile
from concourse.bass_utils import run_bass_kernel_spmd

HOPS, B, L, V, D = 3, 64, 50, 10000, 256
NCORES = 8
BL = B * L                # 3200 moving rows
NF = 2 * HOPS * D         # 1536 fused output cols: [Wa0|Wa1|Wa2|Wc0|Wc1|Wc2]
VSH = V // NCORES         # 1250 vocab rows per core
KT = 10                   # contraction tiles of 128 per core
VPAD = KT * 128           # 1280 (zero-padded)
# Moving-col chunks: each (128, w) fp32 PSUM tile must stay inside one 2KB
# bank, i.e. w <= 512.  Wider chunks stream slightly faster per row.
CHUNKS = [512, 512, 512, 512, 512, 320, 320]
CSTART = [sum(CHUNKS[:i]) for i in range(len(CHUNKS))]
assert sum(CHUNKS) == BL and max(CHUNKS) <= 512
NM = len(CHUNKS)
NN = NF // 128            # 12 stationary W tiles
F16 = mybir.dt.float16
F32 = mybir.dt.float32

_nc_cache = None
_last_result = None       # BassKernelResults of the most recent run (for profiling)


def _legalize_sync(nc):
    """Split multi-wait sync_info into standalone single-wait EventSemaphores.

    The walrus build in this environment enforces the raw-bass contract of at
    most ONE SyncWait per instruction ("Too many sync wait commands" in
    setupSyncWait otherwise), while Tile attaches every needed wait to the
    consuming instruction.  Hoisting all-but-one wait onto preceding
    InstEventSemaphore instructions on the same engine queue is semantically
    identical: engine queues are in-order, so a preceding wait blocks the
    queue exactly like an attached wait.  Updates are left untouched (they
    fire at completion and cannot be hoisted).
    """
    for func in nc.m.functions:
        for block in func.blocks:
            insts = list(block.instructions)
            out = []
            n = 0
            for inst in insts:
                si = inst.sync_info
                if si is not None and len(si.on_wait) > 1:
                    waits = list(si.on_wait)
                    for w in waits[:-1]:
                        ev = mybir.InstEventSemaphore(
                            name=f"{inst.name}-hoistw{n}", ins=[], outs=[]
                        )
                        n += 1
                        ev.engine = inst.engine
                        ev.sync_info = mybir.SyncInfo(on_wait=[w], on_update=[])
                        nc.register_instruction(ev)
                        out.append(ev)
                    inst.sync_info = mybir.SyncInfo(
                        on_wait=[waits[-1]], on_update=list(si.on_update)
                    )
                out.append(inst)
            if len(out) != len(insts):
                block.instructions = out
    return nc


def _dedup_ldweights(nc):
    """Drop InstLdweights that reload the exact weights already in the PE.

    The Tile pipeline splits every matmul into (InstLdweights, InstMatmult
    [non-self-loading]) and emits one Ldweights per matmul even when
    consecutive matmuls share the same stationary tile.  Weights persist in
    the array across matmuls, so a Ldweights whose access pattern equals the
    previous one on the same queue is a pure ~128-cycle waste on the PE's
    critical path (measured ~336 vs ~200 cyc/matmul at 400 moving cols).
    Any sync waits/updates on a dropped Ldweights are merged into the next
    instruction so the Tile dependency tracking stays intact.
    """

    def key(inst):
        ap = inst.ins[0]
        return (
            ap.memref,
            ap.offset,
            str(ap.ap),
            str(ap.dtype),
            getattr(inst, "is_transpose", None),
            getattr(inst, "perf_mode", None),
        )

    dropped = 0
    for func in nc.m.functions:
        for block in func.blocks:
            insts = list(block.instructions)
            out = []
            last_key = None
            pending_sync = []  # sync_infos of dropped ldweights
            for inst in insts:
                if isinstance(inst, mybir.InstLdweights):
                    k = key(inst)
                    if k == last_key:
                        if inst.sync_info is not None:
                            pending_sync.append(inst.sync_info)
                        dropped += 1
                        continue
                    last_key = k
                elif isinstance(inst, mybir.InstMatmult):
                    if getattr(inst, "ldweights", False):
                        last_key = None  # self-loading matmul clobbers array
                if pending_sync:
                    waits = list(inst.sync_info.on_wait) if inst.sync_info else []
                    updates = list(inst.sync_info.on_update) if inst.sync_info else []
                    for si in pending_sync:
                        waits.extend(si.on_wait)
                        updates.extend(si.on_update)
                    inst.sync_info = mybir.SyncInfo(on_wait=waits, on_update=updates)
                    pending_sync = []
                out.append(inst)
            assert not pending_sync, "dropped ldweights sync with no successor"
            if len(out) != len(insts):
                block.instructions = out
    return dropped


def _build(reps=1):
    """Build the SPMD device program.

    reps>1 repeats the main loop body (same data, same output addresses) —
    used only by the benchmark harness to measure device time differentially
    (per-call dispatch noise over the axon tunnel is ~ms, device time is
    ~200 us, so wall-clocking one launch cannot resolve it).
    """
    nc = bass.Bass(trn_type="TRN2")
    facts_t = nc.dram_tensor("facts_t", [VPAD, BL], F16, kind="ExternalInput")
    wac = nc.dram_tensor("wac", [VPAD, NF], F16, kind="ExternalInput")
    q_t = nc.dram_tensor("q_t", [VPAD, B], F16, kind="ExternalInput")
    wq = nc.dram_tensor("wq", [VPAD, D], F16, kind="ExternalInput")
    pac_t = nc.dram_tensor("pac_t", [NF, BL], F16, kind="ExternalOutput")
    pu = nc.dram_tensor("pu", [B, D], F32, kind="ExternalOutput")

    fr = facts_t.rearrange("(k p) n -> p k n", p=128)
    wr = wac.rearrange("(k p) n -> p k n", p=128)
    qr = q_t.rearrange("(k p) n -> p k n", p=128)
    wqr = wq.rearrange("(k p) n -> p k n", p=128)

    with (
        tile.TileContext(nc) as tc,
        tc.tile_pool(name="wpool", bufs=1) as wpool,
        tc.tile_pool(name="opool", bufs=8) as opool,
        tc.tile_pool(name="pspool", bufs=8, space="PSUM") as pspool,
    ):
        # Prologue DMA order: the n=0 wac slice (all k), then facts k-slice
        # major (k=0 unblocks the first 8 matmuls), then the rest of wac,
        # then the small question tensors.
        wt = wpool.tile([128, KT, NF], F16)
        nc.sync.dma_start(wt[:, :, 0:128], wr[:, :, 0:128])
        xt = wpool.tile([128, KT, BL], F16)
        for k in range(KT):
            nc.sync.dma_start(xt[:, k, :], fr[:, k, :])
        for off in range(128, NF, 512):
            end = min(off + 512, NF)
            nc.sync.dma_start(wt[:, :, off:end], wr[:, :, off:end])
        qtile = wpool.tile([128, KT, B], F16)
        nc.sync.dma_start(qtile[:], qr)
        wqt = wpool.tile([128, KT, D], F16)
        nc.sync.dma_start(wqt[:], wqr)

        # Main fused matmul: out(n, m) += sum_k wac[k, n].T @ facts_t[k, m].
        # One stationary tile (k, n) feeds all 8 moving chunks; the 8 chunks
        # of one n-group occupy all 8 PSUM banks; drains interleave with the
        # closing k=KT-1 matmuls, alternating DVE/ACT.
        for _ in range(reps):
            for n in range(NN):
                pss = [
                    pspool.tile([128, CHUNKS[mi]], F32, tag="ps", name="ps",
                                padded_shape=[128, 512])
                    for mi in range(NM)
                ]
                for k in range(KT):
                    last = k == KT - 1
                    for mi in range(NM):
                        nc.tensor.matmul(
                            pss[mi][:],
                            wt[:, k, n * 128 : (n + 1) * 128],
                            xt[:, k, CSTART[mi] : CSTART[mi] + CHUNKS[mi]],
                            start=(k == 0),
                            stop=last,
                        )
                        if last:
                            ot = opool.tile([128, CHUNKS[mi]], F16, tag="ot",
                                            name="ot", padded_shape=[128, 512])
                            if mi % 2 == 0:
                                nc.vector.tensor_copy(ot[:], pss[mi][:])
                            else:
                                nc.scalar.copy(out=ot[:], in_=pss[mi][:])
                            nc.sync.dma_start(
                                pac_t[
                                    n * 128 : (n + 1) * 128,
                                    CSTART[mi] : CSTART[mi] + CHUNKS[mi],
                                ],
                                ot[:],
                            )

        # Question embedding at the tail: its PE work (10 small matmuls)
        # overlaps the main loop's epilogue.
        psq = pspool.tile([B, D], F32, tag="ps", name="ps", padded_shape=[128, 512])
        for k in range(KT):
            nc.tensor.matmul(
                psq[:], qtile[:, k, :], wqt[:, k, :], start=(k == 0), stop=(k == KT - 1)
            )
        uo = opool.tile([B, D], F32, tag="uo")
        nc.any.tensor_copy(out=uo[:], in_=psq[:])
        nc.sync.dma_start(pu[:, :], uo[:])
    _dedup_ldweights(nc)
    return _legalize_sync(nc)


def _shard_inputs(facts, question, Wq, Wa, Wc):
    fx = np.ascontiguousarray(facts, dtype=np.float32).reshape(BL, V)
    qx = np.asarray(question, dtype=np.float32).sum(axis=1)  # (B, V) bag-of-words
    Wq = np.asarray(Wq, dtype=np.float32)
    Wa = np.asarray(Wa, dtype=np.float32)
    Wc = np.asarray(Wc, dtype=np.float32)
    wac_full = np.concatenate([Wa[0], Wa[1], Wa[2], Wc[0], Wc[1], Wc[2]], axis=1)

    in_maps = []
    for c in range(NCORES):
        sl = slice(c * VSH, (c + 1) * VSH)
        ft = np.zeros((VPAD, BL), np.float16)
        ft[:VSH] = fx[:, sl].T
        qt = np.zeros((VPAD, B), np.float16)
        qt[:VSH] = qx[:, sl].T
        ws = np.zeros((VPAD, NF), np.float16)
        ws[:VSH] = wac_full[sl]
        wqs = np.zeros((VPAD, D), np.float16)
        wqs[:VSH] = Wq[sl]
        in_maps.append({"facts_t": ft, "q_t": qt, "wac": ws, "wq": wqs})
    return in_maps


def _wait_for_devices(min_wait_attempts=10):
    """The axon terminal occasionally reports a transient bad topology
    ("terminal has 1 core"); poll until all 8 NeuronCores are visible."""
    import time as _time

    import jax

    for attempt in range(min_wait_attempts):
        try:
            if len(jax.devices()) >= NCORES:
                return
        except Exception:  # noqa: BLE001 - backend init failure is retryable
            try:
                jax.clear_backends()
            except Exception:  # noqa: BLE001
                pass
        _time.sleep(15.0)
    # fall through: let the run itself raise a descriptive error


def _run_with_retries(nc, in_maps, attempts=4):
    """run_bass_kernel_spmd with retries: the axon terminal occasionally
    reports transient failures (device wedged / NRT_EXEC_UNIT_UNRECOVERABLE /
    temporary topology glitches) that succeed on re-dispatch."""
    import time as _time

    last_exc = None
    for attempt in range(attempts):
        try:
            return run_bass_kernel_spmd(nc, in_maps, list(range(NCORES)))
        except Exception as e:  # noqa: BLE001 - retry any runtime failure
            last_exc = e
            if attempt < attempts - 1:
                _time.sleep(10.0 * (attempt + 1))
                _wait_for_devices(min_wait_attempts=4)
    raise last_exc


def kernel(facts, question, Wq, Wa, Wc, Ww, bw):
    global _nc_cache, _last_result
    _wait_for_devices(min_wait_attempts=8)
    in_maps = _shard_inputs(facts, question, Wq, Wa, Wc)
    if _nc_cache is None:
        _nc_cache = _build()
    _last_result = _run_with_retries(_nc_cache, in_maps)
    res = _last_result.results

    # Unshard: sum the 8 partial products of the vocab-sharded matmul.
    ac_t = res[0]["pac_t"].astype(np.float32)
    u = res[0]["pu"].copy()
    for r in res[1:]:
        ac_t += r["pac_t"].astype(np.float32)
        u += r["pu"]

    # Sequential hop recurrence (tiny: ~30 MFLOP vs 98.3 GFLOP on device).
    Ww = np.asarray(Ww, dtype=np.float32)
    bw = np.asarray(bw, dtype=np.float32)
    for h in range(HOPS):
        A = ac_t[h * D : (h + 1) * D].reshape(D, B, L)
        C = ac_t[(HOPS + h) * D : (HOPS + h + 1) * D].reshape(D, B, L)
        match = np.einsum("dbl,bd->bl", A, u)
        mm = match - match.max(axis=-1, keepdims=True)
        e = np.exp(mm)
        p = e / e.sum(axis=-1, keepdims=True)
        att = np.einsum("bl,dbl->bd", p, C)
        z = (u + att) @ Ww[h] + bw[h]
        if h == HOPS - 1:
            zz = z - z.max(axis=-1, keepdims=True)
            ez = np.exp(zz)
            u = ez / ez.sum(axis=-1, keepdims=True)
        else:
            u = np.maximum(z, 0.0)
    return np.ascontiguousarray(u, dtype=np.float32)
